# revision 5
# baseline (speedup 1.0000x reference)
"""EpropGateL0rd fused kernel for 8 TRN2 NeuronCores.

Data-parallel over the batch dim (B=128 -> 16 per core); weights replicated.
Per-core bass kernel:
  phase 1: g/r/p/o matmuls in transposed layout [h,b] (gate values on
           partitions), activations on ScalarE, coefficient vectors
           (1-g), dg*delta_h, dr*g as [128,4,16] tiles.
  phase 2: stream the four [16,512,512] eligibility traces; per (trace,b)
           one 1MB load, rank-1 PE broadcast of x[b]/h[b] across partitions,
           outer products via per-partition scalar multiply (ScalarE from
           PSUM / GpSimd from SBUF), fused update
           e_new = (e * (1-g)[h]) + outer in one VectorE scalar_tensor_tensor,
           one 1MB store.
"""

import os
import numpy as np

B, NI, NH, NO = 128, 512, 512, 512
NCORES = 8
BC = B // NCORES  # 16
KT = NH // 128    # 4 k/h tiles

_CACHE = {}

W_NAMES = ["wgx", "wgh", "wrx", "wrh", "wpx", "wph", "wox", "woh"]
TRACES = ["gx", "gh", "rx", "rh"]


def _build_nc():
    from contextlib import ExitStack

    import concourse.bass as bass
    import concourse.tile as tile
    from concourse import bacc, mybir

    f32 = mybir.dt.float32
    AF = mybir.ActivationFunctionType
    OP = mybir.AluOpType
    AX = mybir.AxisListType

    nc = bacc.Bacc("TRN2", target_bir_lowering=False, debug=False,
                   num_devices=NCORES)

    def din(name, shape):
        return nc.dram_tensor(name, shape, f32, kind="ExternalInput").ap()

    def dout(name, shape):
        return nc.dram_tensor(name, shape, f32, kind="ExternalOutput").ap()

    d_xT = din("xT", [NI, BC])
    d_hT = din("hT", [NH, BC])
    d_xrow = din("xrow", [BC, NI])
    d_hrow = din("hrow", [BC, NH])
    d_w = {n: din(n + "T", [NI, NH]) for n in W_NAMES}
    d_bg = din("bg", [128, KT])
    d_br = din("br", [128, KT])
    d_bp = din("bp", [128, KT])
    d_bo = din("bo", [128, KT])
    d_e = {t: din("e_" + t, [BC, KT, 128, 512]) for t in TRACES}
    d_ebgT = din("ebgT", [NH, BC])
    d_ebrT = din("ebrT", [NH, BC])

    d_outT = dout("outT", [NO, BC])
    d_hnT = dout("hnT", [NH, BC])
    d_en = {t: dout("en_" + t, [BC, KT, 128, 512]) for t in TRACES}
    d_ebgnT = dout("ebgnT", [NH, BC])
    d_ebrnT = dout("ebrnT", [NH, BC])
    d_hg = dout("hg", [128, KT])

    with tile.TileContext(nc) as tc, ExitStack() as ctx:
        const = ctx.enter_context(tc.tile_pool(name="const", bufs=1))
        small = ctx.enter_context(tc.tile_pool(name="small", bufs=1))
        tmp = ctx.enter_context(tc.tile_pool(name="tmp", bufs=2))
        psum = ctx.enter_context(tc.tile_pool(name="psum", bufs=2, space="PSUM"))
        bigin = ctx.enter_context(tc.tile_pool(name="bigin", bufs=4))
        bigout = ctx.enter_context(tc.tile_pool(name="bigout", bufs=3))
        opool = ctx.enter_context(tc.tile_pool(name="outer", bufs=3))

        # ---- constant / input loads --------------------------------------
        w_sb = {}
        for n in W_NAMES:
            tiles = []
            for k in range(KT):
                t = const.tile([128, NH], f32, tag=f"w_{n}{k}")
                nc.sync.dma_start(t[:, :], d_w[n][k * 128:(k + 1) * 128, :])
                tiles.append(t)
            w_sb[n] = tiles

        xT = const.tile([128, KT, BC], f32, tag="xT")
        nc.sync.dma_start(xT[:, :, :], d_xT.rearrange("(k p) b -> p k b", p=128))
        hT = const.tile([128, KT, BC], f32, tag="hT")
        nc.sync.dma_start(hT[:, :, :], d_hT.rearrange("(k p) b -> p k b", p=128))
        bias = {}
        for nm, d in (("g", d_bg), ("r", d_br), ("p", d_bp), ("o", d_bo)):
            t = const.tile([128, KT], f32, tag=f"b{nm}")
            nc.sync.dma_start(t[:, :], d[:, :])
            bias[nm] = t
        ebgT = const.tile([128, KT, BC], f32, tag="ebgT")
        nc.sync.dma_start(ebgT[:, :, :], d_ebgT.rearrange("(k p) b -> p k b", p=128))
        ebrT = const.tile([128, KT, BC], f32, tag="ebrT")
        nc.sync.dma_start(ebrT[:, :, :], d_ebrT.rearrange("(k p) b -> p k b", p=128))

        # ---- phase 1: gates, candidate, new state, coefficients ----------
        gT = small.tile([128, KT, BC], f32, tag="gT")
        rT = small.tile([128, KT, BC], f32, tag="rT")
        omg = small.tile([128, KT, BC], f32, tag="omg")      # 1 - g
        a1 = small.tile([128, KT, BC], f32, tag="a1")        # dg * delta_h
        a2 = small.tile([128, KT, BC], f32, tag="a2")        # dr * g
        hnew = small.tile([128, KT, BC], f32, tag="hnew")
        hg_sums = small.tile([128, KT], f32, tag="hg")
        ebgn = small.tile([128, KT, BC], f32, tag="ebgn")
        ebrn = small.tile([128, KT, BC], f32, tag="ebrn")

        for ht in range(KT):
            hs = slice(ht * 128, (ht + 1) * 128)
            pg = psum.tile([128, BC], f32, tag="mm", bufs=4)
            for k in range(KT):
                nc.tensor.matmul(pg[:, :], w_sb["wgx"][k][:, hs], xT[:, k, :],
                                 start=(k == 0), stop=False)
            for k in range(KT):
                nc.tensor.matmul(pg[:, :], w_sb["wgh"][k][:, hs], hT[:, k, :],
                                 start=False, stop=(k == KT - 1))
            pr = psum.tile([128, BC], f32, tag="mm", bufs=4)
            for k in range(KT):
                nc.tensor.matmul(pr[:, :], w_sb["wrx"][k][:, hs], xT[:, k, :],
                                 start=(k == 0), stop=False)
            for k in range(KT):
                nc.tensor.matmul(pr[:, :], w_sb["wrh"][k][:, hs], hT[:, k, :],
                                 start=False, stop=(k == KT - 1))

            gtanh = tmp.tile([128, BC], f32, tag="gtanh")
            nc.scalar.activation(gtanh[:, :], pg[:, :], AF.Tanh,
                                 bias=bias["g"][:, ht:ht + 1])
            nc.vector.tensor_scalar_max(gT[:, ht, :], gtanh[:, :], 0.0)
            nc.scalar.activation(rT[:, ht, :], pr[:, :], AF.Tanh,
                                 bias=bias["r"][:, ht:ht + 1])

            # 1 - g
            nc.vector.tensor_scalar(omg[:, ht, :], gT[:, ht, :], -1.0, 1.0,
                                    OP.mult, OP.add)
            # delta = r - h_last
            delta = tmp.tile([128, BC], f32, tag="delta")
            nc.vector.tensor_sub(delta[:, :], rT[:, ht, :], hT[:, ht, :])
            # h_new = h + g*delta
            gd = tmp.tile([128, BC], f32, tag="gd")
            nc.vector.tensor_mul(gd[:, :], gT[:, ht, :], delta[:, :])
            nc.vector.tensor_add(hnew[:, ht, :], hT[:, ht, :], gd[:, :])
            # H_g = g > 0 ; accumulate openings partial sums
            Hg = tmp.tile([128, BC], f32, tag="Hg")
            nc.vector.tensor_scalar(Hg[:, :], gT[:, ht, :], 0.0, None, OP.is_gt)
            nc.vector.tensor_reduce(hg_sums[:, ht:ht + 1], Hg[:, :], AX.X, OP.add)
            # dg = (1 - g^2) * Hg
            gsq = tmp.tile([128, BC], f32, tag="gsq")
            nc.vector.tensor_mul(gsq[:, :], gT[:, ht, :], gT[:, ht, :])
            omsq = tmp.tile([128, BC], f32, tag="omsq")
            nc.vector.tensor_scalar(omsq[:, :], gsq[:, :], -1.0, 1.0,
                                    OP.mult, OP.add)
            dg = tmp.tile([128, BC], f32, tag="dg")
            nc.vector.tensor_mul(dg[:, :], omsq[:, :], Hg[:, :])
            # a1 = dg * delta
            nc.vector.tensor_mul(a1[:, ht, :], dg[:, :], delta[:, :])
            # dr = 1 - r^2 ; a2 = dr * g
            rsq = tmp.tile([128, BC], f32, tag="rsq")
            nc.vector.tensor_mul(rsq[:, :], rT[:, ht, :], rT[:, ht, :])
            omr = tmp.tile([128, BC], f32, tag="omr")
            nc.vector.tensor_scalar(omr[:, :], rsq[:, :], -1.0, 1.0,
                                    OP.mult, OP.add)
            nc.vector.tensor_mul(a2[:, ht, :], omr[:, :], gT[:, ht, :])
            # bias-trace updates: e*(1-g) + a
            t1 = tmp.tile([128, BC], f32, tag="t1")
            nc.vector.tensor_mul(t1[:, :], ebgT[:, ht, :], omg[:, ht, :])
            nc.vector.tensor_add(ebgn[:, ht, :], t1[:, :], a1[:, ht, :])
            t2 = tmp.tile([128, BC], f32, tag="t2")
            nc.vector.tensor_mul(t2[:, :], ebrT[:, ht, :], omg[:, ht, :])
            nc.vector.tensor_add(ebrn[:, ht, :], t2[:, :], a2[:, ht, :])

        nc.scalar.dma_start(d_hnT.rearrange("(k p) b -> p k b", p=128),
                            hnew[:, :, :])
        nc.scalar.dma_start(d_ebgnT.rearrange("(k p) b -> p k b", p=128),
                            ebgn[:, :, :])
        nc.scalar.dma_start(d_ebrnT.rearrange("(k p) b -> p k b", p=128),
                            ebrn[:, :, :])
        nc.scalar.dma_start(d_hg[:, :], hg_sums[:, :])

        # output projection p/o (needs full hnew)
        outT = small.tile([128, KT, BC], f32, tag="outT")
        for ot in range(KT):
            os_ = slice(ot * 128, (ot + 1) * 128)
            pp = psum.tile([128, BC], f32, tag="mm", bufs=4)
            for k in range(KT):
                nc.tensor.matmul(pp[:, :], w_sb["wpx"][k][:, os_], xT[:, k, :],
                                 start=(k == 0), stop=False)
            for k in range(KT):
                nc.tensor.matmul(pp[:, :], w_sb["wph"][k][:, os_], hnew[:, k, :],
                                 start=False, stop=(k == KT - 1))
            po = psum.tile([128, BC], f32, tag="mm", bufs=4)
            for k in range(KT):
                nc.tensor.matmul(po[:, :], w_sb["wox"][k][:, os_], xT[:, k, :],
                                 start=(k == 0), stop=False)
            for k in range(KT):
                nc.tensor.matmul(po[:, :], w_sb["woh"][k][:, os_], hnew[:, k, :],
                                 start=False, stop=(k == KT - 1))
            pT = tmp.tile([128, BC], f32, tag="pT")
            nc.scalar.activation(pT[:, :], pp[:, :], AF.Tanh,
                                 bias=bias["p"][:, ot:ot + 1])
            oT = tmp.tile([128, BC], f32, tag="oT")
            nc.scalar.activation(oT[:, :], po[:, :], AF.Sigmoid,
                                 bias=bias["o"][:, ot:ot + 1])
            nc.vector.tensor_mul(outT[:, ot, :], oT[:, :], pT[:, :])
        nc.scalar.dma_start(d_outT.rearrange("(k p) b -> p k b", p=128),
                            outT[:, :, :])

        # ---- phase 2: eligibility-trace streams --------------------------
        for b in range(BC):
            # broadcast x[b,:] / h[b,:] across partitions via 0-stride DMA
            xb = tmp.tile([128, NI], f32, tag="xb", bufs=2)
            nc.sync.dma_start(xb[:, :], d_xrow[b].partition_broadcast(128))
            hb = tmp.tile([128, NH], f32, tag="hb", bufs=2)
            nc.sync.dma_start(hb[:, :], d_hrow[b].partition_broadcast(128))

            for t, coef, vec, on_act in (("gx", a1, xb, True),
                                         ("gh", a1, hb, False),
                                         ("rx", a2, xb, True),
                                         ("rh", a2, hb, False)):
                ein = bigin.tile([128, KT, 512], f32, tag="ein")
                nc.sync.dma_start(ein[:, :, :], d_e[t][b].transpose([1, 0, 2]))
                outer = opool.tile([128, KT, 512], f32, tag="outer")
                eout = bigout.tile([128, KT, 512], f32, tag="eout")
                for ht in range(KT):
                    c_ap = coef[:, ht, b:b + 1]
                    if on_act:
                        nc.scalar.activation(outer[:, ht, :], vec[:, :],
                                             AF.Copy, scale=c_ap)
                    else:
                        nc.gpsimd.tensor_scalar_mul(outer[:, ht, :], vec[:, :],
                                                    c_ap)
                    nc.vector.scalar_tensor_tensor(
                        eout[:, ht, :], ein[:, ht, :], omg[:, ht, b:b + 1],
                        outer[:, ht, :], OP.mult, OP.add)
                nc.scalar.dma_start(d_en[t][b].transpose([1, 0, 2]),
                                    eout[:, :, :])

    nc.compile()
    return nc


def _get_nc():
    if "nc" not in _CACHE:
        _CACHE["nc"] = _build_nc()
    return _CACHE["nc"]


def _make_in_maps(x, h_last, w, biases, e, e_b_g, e_b_r):
    f32 = np.float32
    asc = np.ascontiguousarray

    wT = {n: asc(w[n].T.astype(f32, copy=False)) for n in W_NAMES}
    bpk = {n: asc(biases[n].astype(f32, copy=False).reshape(KT, 128).T)
           for n in ("g", "r", "p", "o")}

    in_maps = []
    for c in range(NCORES):
        sl = slice(c * BC, (c + 1) * BC)
        xc = asc(x[sl].astype(f32, copy=False))
        hc = asc(h_last[sl].astype(f32, copy=False))
        m = {
            "xT": asc(xc.T), "hT": asc(hc.T), "xrow": xc, "hrow": hc,
            "bg": bpk["g"], "br": bpk["r"], "bp": bpk["p"], "bo": bpk["o"],
            "ebgT": asc(e_b_g[sl].astype(f32, copy=False).T),
            "ebrT": asc(e_b_r[sl].astype(f32, copy=False).T),
        }
        for n in W_NAMES:
            m[n + "T"] = wT[n]
        for t in TRACES:
            m["e_" + t] = asc(e[t][sl].astype(f32, copy=False)).reshape(
                BC, KT, 128, 512)
        in_maps.append(m)
    return in_maps


def _make_in_maps_from(inputs):
    i = {k: np.asarray(v) for k, v in inputs.items()}
    return _make_in_maps(
        i["x"], i["h_last"],
        {"wgx": i["w_gx"], "wgh": i["w_gh"], "wrx": i["w_rx"],
         "wrh": i["w_rh"], "wpx": i["w_px"], "wph": i["w_ph"],
         "wox": i["w_ox"], "woh": i["w_oh"]},
        {"g": i["b_g"], "r": i["b_r"], "p": i["b_p"], "o": i["b_o"]},
        {"gx": i["e_w_gx"], "gh": i["e_w_gh"], "rx": i["e_w_rx"],
         "rh": i["e_w_rh"]},
        i["e_b_g"], i["e_b_r"])


def kernel(x, h_last, w_gx, w_gh, b_g, w_rx, w_rh, b_r,
           w_px, w_ph, b_p, w_ox, w_oh, b_o,
           e_w_gx, e_w_gh, e_b_g, e_w_rx, e_w_rh, e_b_r):
    from concourse.bass_utils import run_bass_kernel_spmd

    nc = _get_nc()
    x = np.asarray(x)
    h_last = np.asarray(h_last)
    in_maps = _make_in_maps(
        x, h_last,
        {"wgx": np.asarray(w_gx), "wgh": np.asarray(w_gh),
         "wrx": np.asarray(w_rx), "wrh": np.asarray(w_rh),
         "wpx": np.asarray(w_px), "wph": np.asarray(w_ph),
         "wox": np.asarray(w_ox), "woh": np.asarray(w_oh)},
        {"g": np.asarray(b_g), "r": np.asarray(b_r),
         "p": np.asarray(b_p), "o": np.asarray(b_o)},
        {"gx": np.asarray(e_w_gx), "gh": np.asarray(e_w_gh),
         "rx": np.asarray(e_w_rx), "rh": np.asarray(e_w_rh)},
        np.asarray(e_b_g), np.asarray(e_b_r))

    res = run_bass_kernel_spmd(nc, in_maps, list(range(NCORES)))
    _CACHE["last_results"] = res
    r = res.results

    cat = np.concatenate
    out = cat([np.ascontiguousarray(r[c]["outT"].T) for c in range(NCORES)], 0)
    h = cat([np.ascontiguousarray(r[c]["hnT"].T) for c in range(NCORES)], 0)
    e_n = {t: cat([r[c]["en_" + t].reshape(BC, NH, NI)
                   for c in range(NCORES)], 0) for t in TRACES}
    e_b_g_n = cat([np.ascontiguousarray(r[c]["ebgnT"].T)
                   for c in range(NCORES)], 0)
    e_b_r_n = cat([np.ascontiguousarray(r[c]["ebrnT"].T)
                   for c in range(NCORES)], 0)
    openings = np.float32(
        sum(float(r[c]["hg"].sum()) for c in range(NCORES)) / (B * NH))

    return (out, h, e_n["gx"], e_n["gh"], e_b_g_n,
            e_n["rx"], e_n["rh"], e_b_r_n, openings)
